# revision 1
# baseline (speedup 1.0000x reference)
"""Trainium2 Bass kernel for nn_LocalRNN: 8-step CTRNN over sliding windows.

Math:
  For each position l: h_{k+1} = a*h_k + relu(h_k @ W* + u*[l+k]),  h_0 = 0
  where a = 1 - 1/tau, W* = W * (1/tau) (columns), u* = Xp @ W_in* + b*,
  W_in* = W_in * (1/tau), b* = b * (1/tau).  Output = h_8 per position.
  (Uses relu(c*z) = c*relu(z) for c>0 to fold 1/tau into the weights, and
  the fact that the input projection is shared across overlapping windows.)

Sharding: batch dim (8) across the 8 NeuronCores, weights replicated.
On-chip layout is transposed ([d on partitions, positions on free dim]) so
matmuls contract d on the partition axis; the host uploads x pre-transposed
and transposes the [d, pos] output back (layout marshalling only).
"""

import numpy as np
from contextlib import ExitStack

import concourse.bass as bass
import concourse.tile as tile
from concourse import bacc, mybir
from concourse.bass_utils import run_bass_kernel_spmd

B, L, D, KSIZE = 8, 2048, 256, 8
P = 128
NCORES = 8
MMN = 512                    # matmul moving free dim (PSUM bank limit)
WCH = 1024                   # wide chunk for ACT/DVE elementwise ops
NW = L // WCH                # 2
NG = L // MMN                # 4 groups of 512
UCOLS = L + KSIZE - 1        # 2055
PAD = KSIZE - 1              # 7
DB = D // P                  # 2 d-blocks
F32 = mybir.dt.float32
F32R = mybir.dt.float32r
AF = mybir.ActivationFunctionType
ALU = mybir.AluOpType

# packed f32r consts blobs: cru = wint0|wint1 ; crw = wt0|wt1|identr
CRU_COLS = 2 * D
CRW_COLS = 2 * D + P
CRW_ID = 2 * D
# packed f32 consts blob: bst|at|pad src
CF_COLS = 2 * DB + P
_cache = {}


def _build_program():
    nc = bacc.Bacc(
        "TRN2",
        target_bir_lowering=False,
        debug=False,
        enable_asserts=False,
        num_devices=NCORES,
    )
    # x uploaded pre-transposed: (D, L), row d -> [d, positions]
    x_d = nc.dram_tensor("xt", (D, L), F32R, kind="ExternalInput").ap()
    cru_d = nc.dram_tensor("constsru", (P, CRU_COLS), F32R, kind="ExternalInput").ap()
    crw_d = nc.dram_tensor("constsrw", (P, CRW_COLS), F32R, kind="ExternalInput").ap()
    cf_d = nc.dram_tensor("constsf", (P, CF_COLS), F32, kind="ExternalInput").ap()
    # output in T-layout: (D, L); host transposes back
    out_d = nc.dram_tensor("out", (D, L), F32R, kind="ExternalOutput").ap()

    with tile.TileContext(nc) as tc, ExitStack() as ctx:
        consts = ctx.enter_context(tc.tile_pool(name="consts", bufs=1))
        big = ctx.enter_context(tc.tile_pool(name="big", bufs=1))
        rp = ctx.enter_context(tc.tile_pool(name="rp", bufs=3))
        # single PSUM pool, all tags share slots: [128,1024] slot = 2 banks,
        # bufs=4 -> 8 banks
        zp = ctx.enter_context(tc.tile_pool(name="zp", bufs=4, space="PSUM"))

        # --- PE warmup: dummy matmuls on garbage data to engage HAM early ---
        dummy_f = big.tile([P, MMN], F32, name="dummy_f")
        dummy = big.tile([P, MMN], F32R, name="dummy")
        nc.vector.memset(dummy_f[:], 0.0)
        nc.vector.tensor_copy(dummy[:], dummy_f[:])
        warm = zp.tile([P, MMN], F32, name="warm", tag="z")
        for _ in range(20):
            nc.tensor.matmul(warm[:], lhsT=dummy[:, 0:P], rhs=dummy[:],
                             start=True, stop=True)

        # --- constants ---
        cru = consts.tile([P, CRU_COLS], F32R, name="cru")
        crw = consts.tile([P, CRW_COLS], F32R, name="crw")
        cf = consts.tile([P, CF_COLS], F32, name="cf")
        wt = [crw[:, i * D:(i + 1) * D] for i in range(DB)]
        wint = [cru[:, i * D:(i + 1) * D] for i in range(DB)]
        identr = crw[:, CRW_ID:CRW_ID + P]
        bst = cf[:, 0:DB]
        at = cf[:, DB:2 * DB]

        # --- persistent buffers ---
        # x in 2 per-position-half tiles (both d-blocks each) so the first
        # half of the u projection only waits on one 1MB DMA
        xth = [big.tile([P, DB * WCH], F32R, name=f"xth{g}") for g in range(2)]
        ut = [big.tile([P, UCOLS], F32R, name=f"ut{i}") for i in range(DB)]
        hball = [big.tile([P, DB * L], F32R, name=f"hb{s}") for s in range(2)]
        hb = [[hball[s][:, i * L:(i + 1) * L] for i in range(DB)]
              for s in range(2)]
        h1 = hb[1]

        # --- input DMAs. Same-engine transfers serialize through one ring;
        # the two engines' rings share HBM BW. Land the first u-chunk's
        # inputs (cf, cru, xth0) before everything else.
        def xdma(eng, g):
            eng.dma_start(
                xth[g][:].rearrange("p (i c) -> p i c", i=DB),
                x_d.rearrange("(i p) c -> p i c", p=P)[
                    :, :, g * WCH:(g + 1) * WCH],
            )
        nc.sync.dma_start(cf[:], cf_d[:, :])
        nc.scalar.dma_start(cru[:], cru_d[:, :])
        xdma(nc.sync, 0)
        xdma(nc.scalar, 1)
        nc.sync.dma_start(crw[:], crw_d[:, :])

        # u pad cols + h1 pad cols (also warms the ACT table early):
        # u[:, :7] = b*, h1[:, :7] = relu(b*)
        for j in range(DB):
            nc.scalar.activation(
                ut[j][:, 0:PAD], cf[:, 0:PAD],
                AF.Identity, bias=bst[:, j:j + 1], scale=0.0,
            )
            nc.scalar.activation(
                h1[j][:, 0:PAD], cf[:, 0:PAD],
                AF.Relu, bias=bst[:, j:j + 1], scale=0.0,
            )


        # --- u projection, wide tiles; h1 (ACT) and u (DVE) read PSUM ---
        for gw in range(2):
            for j in range(DB):
                zt = zp.tile([P, WCH], F32, name="zu", tag="z")
                for half in range(2):
                    g = 2 * gw + half
                    zh = zt[:, half * MMN:(half + 1) * MMN]
                    for i in range(DB):
                        nc.tensor.matmul(
                            zh,
                            lhsT=wint[i][:, j * P:(j + 1) * P],
                            rhs=xth[gw][:, i * WCH + half * MMN:
                                        i * WCH + half * MMN + MMN],
                            start=(i == 0),
                            stop=(i == DB - 1),
                        )
                # h1 positions [7+1024gw, min(7+1024(gw+1), 2048))
                hw = WCH if gw == 0 else WCH - PAD
                nc.scalar.activation(
                    h1[j][:, PAD + gw * WCH:PAD + gw * WCH + hw],
                    zt[:, 0:hw], AF.Relu, bias=bst[:, j:j + 1], scale=1.0,
                )
                nc.vector.tensor_scalar(
                    out=ut[j][:, PAD + gw * WCH:PAD + (gw + 1) * WCH],
                    in0=zt[:],
                    scalar1=bst[:, j:j + 1],
                    scalar2=None,
                    op0=ALU.add,
                )

        # --- steps 1..6 (wide 1024-col chunks; matmuls in 512 halves) ---
        for k in range(1, KSIZE - 1):
            hc = hb[k % 2]
            hn = hb[(k + 1) % 2]
            for c in range(NW):
                cs = c * WCH
                for j in range(DB):
                    zt = zp.tile([P, WCH], F32, name="zt", tag="z")
                    for h in range(2):
                        hs = cs + h * MMN
                        zh = zt[:, h * MMN:(h + 1) * MMN]
                        for i in range(DB):
                            nc.tensor.matmul(
                                zh,
                                lhsT=wt[i][:, j * P:(j + 1) * P],
                                rhs=hc[i][:, hs:hs + MMN],
                                start=(i == 0),
                                stop=False,
                            )
                        nc.tensor.matmul(
                            zh,
                            lhsT=identr,
                            rhs=ut[j][:, k + hs:k + hs + MMN],
                            start=False,
                            stop=True,
                        )
                    r = rp.tile([P, WCH], F32, name="r", tag="r")
                    nc.scalar.activation(r[:], zt[:], AF.Relu)
                    nc.vector.scalar_tensor_tensor(
                        out=hn[j][:, cs:cs + WCH],
                        in0=hc[j][:, cs:cs + WCH],
                        scalar=at[:, j:j + 1],
                        in1=r[:],
                        op0=ALU.mult,
                        op1=ALU.add,
                    )

        # --- step 7 in 512-col chunks, output DMA per chunk ---
        k = KSIZE - 1
        hc = hb[k % 2]
        hn = hb[(k + 1) % 2]
        h8all = hball[(k + 1) % 2]
        for g in range(NG):
            cs = g * MMN
            for j in range(DB):
                zt = zp.tile([P, MMN], F32, name="z7", tag="z")
                for i in range(DB):
                    nc.tensor.matmul(
                        zt[:],
                        lhsT=wt[i][:, j * P:(j + 1) * P],
                        rhs=hc[i][:, cs:cs + MMN],
                        start=(i == 0),
                        stop=False,
                    )
                nc.tensor.matmul(
                    zt[:],
                    lhsT=identr,
                    rhs=ut[j][:, k + cs:k + cs + MMN],
                    start=False,
                    stop=True,
                )
                r = rp.tile([P, MMN], F32, name="r7", tag="r")
                nc.scalar.activation(r[:], zt[:], AF.Relu)
                nc.vector.scalar_tensor_tensor(
                    out=hn[j][:, cs:cs + MMN],
                    in0=hc[j][:, cs:cs + MMN],
                    scalar=at[:, j:j + 1],
                    in1=r[:],
                    op0=ALU.mult,
                    op1=ALU.add,
                )
            eng = nc.sync if g % 2 == 0 else nc.scalar
            eng.dma_start(
                out_d.rearrange("(i p) c -> p i c", p=P)[
                    :, :, cs:cs + MMN],
                h8all[:].rearrange("p (i c) -> p i c", i=DB)[
                    :, :, cs:cs + MMN],
            )

    nc.compile()
    return nc


def get_program():
    if "nc" not in _cache:
        _cache["nc"] = _build_program()
    return _cache["nc"]


def make_in_maps(x, weight, input_weight, bias, tau):
    x = np.asarray(x, dtype=np.float32)
    weight = np.asarray(weight, dtype=np.float32)
    input_weight = np.asarray(input_weight, dtype=np.float32)
    bias = np.asarray(bias, dtype=np.float32).reshape(1, D)
    tau = np.asarray(tau, dtype=np.float32).reshape(1, D)

    inv_tau = 1.0 / tau                       # (1, D)
    a = 1.0 - inv_tau
    wstar = (weight * inv_tau).astype(np.float32)          # scale columns
    winstar = (input_weight * inv_tau).astype(np.float32)
    bstar = (bias * inv_tau).astype(np.float32)
    # per-partition layout (P, DB): col j holds elems [j*P, (j+1)*P)
    bstar_t = bstar.reshape(DB, P).T
    a_t = a.reshape(DB, P).T
    ident = np.eye(P, dtype=np.float32)

    cru = np.concatenate([winstar[0:P, :], winstar[P:D, :]], axis=1)
    crw = np.concatenate([wstar[0:P, :], wstar[P:D, :], ident], axis=1)
    cf = np.concatenate([bstar_t, a_t, np.zeros((P, P), np.float32)], axis=1)

    shared = {
        "constsru": np.ascontiguousarray(cru),
        "constsrw": np.ascontiguousarray(crw),
        "constsf": np.ascontiguousarray(cf),
    }
    return [
        {"xt": np.ascontiguousarray(x[b].T), **shared} for b in range(NCORES)
    ]


def kernel(x, weight, input_weight, bias, tau, ksize, _trace=False):
    assert int(ksize) == KSIZE
    nc = get_program()
    in_maps = make_in_maps(x, weight, input_weight, bias, tau)
    res = run_bass_kernel_spmd(
        nc, in_maps, core_ids=list(range(NCORES)), trace=_trace
    )
    out = np.stack(
        [np.ascontiguousarray(res.results[b]["out"].T) for b in range(NCORES)],
        axis=0,
    )
    if _trace:
        _cache["last_results"] = res
    return out.astype(np.float32)



# revision 2
# speedup vs baseline: 1.2511x; 1.2511x over previous
"""Trainium2 Bass kernel for nn_LocalRNN: 8-step CTRNN over sliding windows.

Math:
  For each position l: h_{k+1} = a*h_k + relu(h_k @ W* + u*[l+k]),  h_0 = 0
  where a = 1 - 1/tau, W* = W * (1/tau) (columns), u* = Xp @ W_in* + b*,
  W_in* = W_in * (1/tau), b* = b * (1/tau).  Output = h_8 per position.
  (Uses relu(c*z) = c*relu(z) for c>0 to fold 1/tau into the weights, and
  the fact that the input projection is shared across overlapping windows.)

Sharding: batch dim (8) across the 8 NeuronCores, weights replicated.
On-chip layout is transposed ([d on partitions, positions on free dim]) so
matmuls contract d on the partition axis; the host uploads x pre-transposed
and swizzled, and un-swizzles the chunk-major output (layout marshalling
only, off the measured path).

All matmul/DVE operands are fp16 (PSUM accumulation stays fp32): enables
FWL weight loads on PE, 2x DVE mode for the recurrence, and halves DMA.
"""

import numpy as np
from contextlib import ExitStack

import concourse.bass as bass
import concourse.tile as tile
from concourse import bacc, mybir
from concourse.bass_utils import run_bass_kernel_spmd

B, L, D, KSIZE = 8, 2048, 256, 8
P = 128
NCORES = 8
MMN = 512                    # matmul moving free dim (PSUM bank limit)
WCH = 1024                   # wide chunk for ACT/DVE elementwise ops
NW = L // WCH                # 2
NG = L // MMN                # 4 groups of 512
UCOLS = L + KSIZE - 1        # 2055
PAD = KSIZE - 1              # 7
DB = D // P                  # 2 d-blocks
F32 = mybir.dt.float32
F16 = mybir.dt.float16
AF = mybir.ActivationFunctionType
ALU = mybir.AluOpType

# fp16 weights blob: wint0|wint1|wt0|wt1|identr
WB_COLS = 4 * D + P
WB_ID = 4 * D
# fp32 consts blob: bst (DB) | at (DB) | spare
CF_COLS = 16
N_WARM = 10
_cache = {}


def _build_program():
    nc = bacc.Bacc(
        "TRN2",
        target_bir_lowering=False,
        debug=False,
        enable_asserts=False,
        num_devices=NCORES,
    )
    # x uploaded pre-transposed+swizzled: row p, cols (i, l): x[l, i*128+p]
    x_d = nc.dram_tensor("xt", (P, DB * L), F16, kind="ExternalInput").ap()
    wb_d = nc.dram_tensor("wblob", (P, WB_COLS), F16, kind="ExternalInput").ap()
    cf_d = nc.dram_tensor("constsf", (P, CF_COLS), F32, kind="ExternalInput").ap()
    # output chunk-major: rows (c, p), cols (i, s): h8[i*128+p, c*512+s]
    out_d = nc.dram_tensor("out", (NG * P, DB * MMN), F16, kind="ExternalOutput").ap()

    with tile.TileContext(nc) as tc, ExitStack() as ctx:
        consts = ctx.enter_context(tc.tile_pool(name="consts", bufs=1))
        big = ctx.enter_context(tc.tile_pool(name="big", bufs=1))
        rp = ctx.enter_context(tc.tile_pool(name="rp", bufs=3))
        # single PSUM pool, all tags share slots: [128,1024] slot = 2 banks,
        # bufs=4 -> 8 banks
        zp = ctx.enter_context(tc.tile_pool(name="zp", bufs=4, space="PSUM"))

        # --- constants ---
        wb = consts.tile([P, WB_COLS], F16, name="wb")
        cf = consts.tile([P, CF_COLS], F32, name="cf")
        wint = [wb[:, i * D:(i + 1) * D] for i in range(DB)]
        wt = [wb[:, 2 * D + i * D:2 * D + (i + 1) * D] for i in range(DB)]
        identr = wb[:, WB_ID:WB_ID + P]
        bst = cf[:, 0:DB]
        at = cf[:, DB:2 * DB]

        # --- persistent buffers ---
        # x in 2 per-position-half tiles (both d-blocks each) so the first
        # half of the u projection only waits on one 0.5MB DMA
        xth = [big.tile([P, DB * WCH], F16, name=f"xth{g}") for g in range(2)]
        ut = [big.tile([P, UCOLS], F16, name=f"ut{i}") for i in range(DB)]
        hball = [big.tile([P, DB * L], F16, name=f"hb{s}") for s in range(2)]
        hb = [[hball[s][:, i * L:(i + 1) * L] for i in range(DB)]
              for s in range(2)]
        h1 = hb[1]

        # --- input DMAs. Only sync+scalar have HWDGE; each dma_start costs
        # ~800ns of descriptor-gen on its sequencer, so spread and order by
        # need: weights first (PE warmup), then cf, then x halves.
        def xdma(eng, g):
            eng.dma_start(
                xth[g][:].rearrange("p (i c) -> p i c", i=DB),
                x_d.rearrange("p (i c) -> p i c", i=DB)[
                    :, :, g * WCH:(g + 1) * WCH],
            )
        nc.sync.dma_start(wb[:], wb_d[:, :])
        nc.scalar.dma_start(cf[:], cf_d[:, :])
        xdma(nc.sync, 0)
        xdma(nc.scalar, 1)

        # --- PE warmup on the weights blob (lands early): engages HAM ramp
        # while x is still in flight. Output garbage to a rotating PSUM slot.
        warm = zp.tile([P, MMN], F32, name="warm", tag="z")
        for _ in range(N_WARM):
            nc.tensor.matmul(warm[:], lhsT=identr, rhs=wb[:, 0:MMN],
                             start=True, stop=True)

        # u pad cols + h1 pad cols (also warms the ACT table early):
        # u[:, :7] = b*, h1[:, :7] = relu(b*)
        for j in range(DB):
            nc.scalar.activation(
                ut[j][:, 0:PAD], cf[:, 0:PAD],
                AF.Identity, bias=bst[:, j:j + 1], scale=0.0,
            )
            nc.scalar.activation(
                h1[j][:, 0:PAD], cf[:, 0:PAD],
                AF.Relu, bias=bst[:, j:j + 1], scale=0.0,
            )

        # --- u projection, wide tiles; h1 (ACT) and u (DVE) read PSUM ---
        for gw in range(2):
            for j in range(DB):
                zt = zp.tile([P, WCH], F32, name="zu", tag="z")
                for half in range(2):
                    g = 2 * gw + half
                    zh = zt[:, half * MMN:(half + 1) * MMN]
                    for i in range(DB):
                        nc.tensor.matmul(
                            zh,
                            lhsT=wint[i][:, j * P:(j + 1) * P],
                            rhs=xth[gw][:, i * WCH + half * MMN:
                                        i * WCH + half * MMN + MMN],
                            start=(i == 0),
                            stop=(i == DB - 1),
                        )
                # h1 positions [7+1024gw, min(7+1024(gw+1), 2048))
                hw = WCH if gw == 0 else WCH - PAD
                nc.scalar.activation(
                    h1[j][:, PAD + gw * WCH:PAD + gw * WCH + hw],
                    zt[:, 0:hw], AF.Relu, bias=bst[:, j:j + 1], scale=1.0,
                )
                nc.vector.tensor_scalar(
                    out=ut[j][:, PAD + gw * WCH:PAD + (gw + 1) * WCH],
                    in0=zt[:],
                    scalar1=bst[:, j:j + 1],
                    scalar2=None,
                    op0=ALU.add,
                )

        # --- steps 1..6 (wide 1024-col chunks; matmuls in 512 halves) ---
        for k in range(1, KSIZE - 1):
            hc = hb[k % 2]
            hn = hb[(k + 1) % 2]
            for c in range(NW):
                cs = c * WCH
                for j in range(DB):
                    zt = zp.tile([P, WCH], F32, name="zt", tag="z")
                    for h in range(2):
                        hs = cs + h * MMN
                        zh = zt[:, h * MMN:(h + 1) * MMN]
                        for i in range(DB):
                            nc.tensor.matmul(
                                zh,
                                lhsT=wt[i][:, j * P:(j + 1) * P],
                                rhs=hc[i][:, hs:hs + MMN],
                                start=(i == 0),
                                stop=False,
                            )
                        nc.tensor.matmul(
                            zh,
                            lhsT=identr,
                            rhs=ut[j][:, k + hs:k + hs + MMN],
                            start=False,
                            stop=True,
                        )
                    r = rp.tile([P, WCH], F16, name="r", tag="r")
                    nc.scalar.activation(r[:], zt[:], AF.Relu)
                    nc.vector.scalar_tensor_tensor(
                        out=hn[j][:, cs:cs + WCH],
                        in0=hc[j][:, cs:cs + WCH],
                        scalar=at[:, j:j + 1],
                        in1=r[:],
                        op0=ALU.mult,
                        op1=ALU.add,
                    )

        # --- step 7 in 512-col chunks, output DMA per chunk ---
        k = KSIZE - 1
        hc = hb[k % 2]
        hn = hb[(k + 1) % 2]
        h8all = hball[(k + 1) % 2]
        for g in range(NG):
            cs = g * MMN
            for j in range(DB):
                zt = zp.tile([P, MMN], F32, name="z7", tag="z")
                for i in range(DB):
                    nc.tensor.matmul(
                        zt[:],
                        lhsT=wt[i][:, j * P:(j + 1) * P],
                        rhs=hc[i][:, cs:cs + MMN],
                        start=(i == 0),
                        stop=False,
                    )
                nc.tensor.matmul(
                    zt[:],
                    lhsT=identr,
                    rhs=ut[j][:, k + cs:k + cs + MMN],
                    start=False,
                    stop=True,
                )
                r = rp.tile([P, MMN], F16, name="r7", tag="r")
                nc.scalar.activation(r[:], zt[:], AF.Relu)
                nc.vector.scalar_tensor_tensor(
                    out=hn[j][:, cs:cs + MMN],
                    in0=hc[j][:, cs:cs + MMN],
                    scalar=at[:, j:j + 1],
                    in1=r[:],
                    op0=ALU.mult,
                    op1=ALU.add,
                )
            # chunk-major store: rows [g*128, (g+1)*128) of out_d, 2KB descs.
            # sync is idle in steady state; keep scalar free for relus.
            eng = nc.sync if g != 1 else nc.scalar
            eng.dma_start(
                out_d.rearrange("(c p) f -> c p f", p=P)[g]
                     .rearrange("p (i s) -> p i s", i=DB),
                h8all[:].rearrange("p (i c) -> p i c", i=DB)[
                    :, :, cs:cs + MMN],
            )

    nc.compile()
    return nc


def get_program():
    if "nc" not in _cache:
        _cache["nc"] = _build_program()
    return _cache["nc"]


def make_in_maps(x, weight, input_weight, bias, tau):
    x = np.asarray(x, dtype=np.float32)
    weight = np.asarray(weight, dtype=np.float32)
    input_weight = np.asarray(input_weight, dtype=np.float32)
    bias = np.asarray(bias, dtype=np.float32).reshape(1, D)
    tau = np.asarray(tau, dtype=np.float32).reshape(1, D)

    inv_tau = 1.0 / tau                       # (1, D)
    a = 1.0 - inv_tau
    wstar = (weight * inv_tau).astype(np.float32)          # scale columns
    winstar = (input_weight * inv_tau).astype(np.float32)
    bstar = (bias * inv_tau).astype(np.float32)
    # per-partition layout (P, DB): col j holds elems [j*P, (j+1)*P)
    bstar_t = bstar.reshape(DB, P).T
    a_t = a.reshape(DB, P).T
    ident = np.eye(P, dtype=np.float32)

    wb = np.concatenate(
        [winstar[0:P, :], winstar[P:D, :], wstar[0:P, :], wstar[P:D, :],
         ident], axis=1).astype(np.float16)
    cf = np.zeros((P, CF_COLS), np.float32)
    cf[:, 0:DB] = bstar_t
    cf[:, DB:2 * DB] = a_t

    shared = {
        "wblob": np.ascontiguousarray(wb),
        "constsf": np.ascontiguousarray(cf),
    }
    ins = []
    for b in range(NCORES):
        # xt[p, i*L + l] = x[b][l, i*128+p]
        xt = np.ascontiguousarray(
            x[b].T.reshape(DB, P, L).transpose(1, 0, 2).reshape(P, DB * L)
            .astype(np.float16))
        ins.append({"xt": xt, **shared})
    return ins


def kernel(x, weight, input_weight, bias, tau, ksize, _trace=False):
    assert int(ksize) == KSIZE
    nc = get_program()
    in_maps = make_in_maps(x, weight, input_weight, bias, tau)
    res = run_bass_kernel_spmd(
        nc, in_maps, core_ids=list(range(NCORES)), trace=_trace
    )
    outs = []
    for b in range(NCORES):
        od = np.asarray(res.results[b]["out"])  # (NG*P, DB*MMN) f16
        out_b = (od.reshape(NG, P, DB, MMN).transpose(0, 3, 2, 1)
                 .reshape(L, D))
        outs.append(out_b)
    out = np.stack(outs, axis=0)
    if _trace:
        _cache["last_results"] = res
    return out.astype(np.float32)


# revision 8
# speedup vs baseline: 1.2865x; 1.0283x over previous
"""Trainium2 Bass kernel for nn_LocalRNN: 8-step CTRNN over sliding windows.

Math:
  For each position l: h_{k+1} = a*h_k + relu(h_k @ W* + u*[l+k]),  h_0 = 0
  where a = 1 - 1/tau, W* = W * (1/tau) (columns), u* = Xp @ W_in* + b*,
  W_in* = W_in * (1/tau), b* = b * (1/tau).  Output = h_8 per position.
  (Uses relu(c*z) = c*relu(z) for c>0 to fold 1/tau into the weights, and
  the fact that the input projection is shared across overlapping windows.)

Sharding: batch dim (8) across the 8 NeuronCores, weights replicated.
On-chip layout is transposed ([d on partitions, positions on free dim]) so
matmuls contract d on the partition axis; the host uploads x pre-transposed
and swizzled, and un-swizzles the chunk-major output (layout marshalling
only, off the measured path).

All matmul/DVE operands are fp16 (PSUM accumulation stays fp32): enables
FWL weight loads on PE, 2x DVE mode for the recurrence, and halves DMA.
"""

import numpy as np
from contextlib import ExitStack

import concourse.bass as bass
import concourse.tile as tile
from concourse import bacc, mybir
from concourse.bass_utils import run_bass_kernel_spmd

B, L, D, KSIZE = 8, 2048, 256, 8
P = 128
NCORES = 8
MMN = 512                    # matmul moving free dim (PSUM bank limit)
WCH = 1024                   # wide chunk for ACT/DVE elementwise ops
NW = L // WCH                # 2
NG = L // MMN                # 4 groups of 512
UCOLS = L + KSIZE - 1        # 2055
PAD = KSIZE - 1              # 7
DB = D // P                  # 2 d-blocks
F32 = mybir.dt.float32
F16 = mybir.dt.float16
AF = mybir.ActivationFunctionType
ALU = mybir.AluOpType

# fp16 weights blob: wint0|wint1|wt0|wt1|identr
WB_COLS = 4 * D + P
WB_ID = 4 * D
# fp32 consts blob: bst (DB) | at (DB) | spare
CF_COLS = 16
N_WARM = 7
_cache = {}


def _build_program():
    nc = bacc.Bacc(
        "TRN2",
        target_bir_lowering=False,
        debug=False,
        enable_asserts=False,
        num_devices=NCORES,
    )
    # x uploaded pre-transposed+swizzled: row p, cols (i, l): x[l, i*128+p]
    x_d = nc.dram_tensor("xt", (P, DB * L), F16, kind="ExternalInput").ap()
    wb_d = nc.dram_tensor("wblob", (P, WB_COLS), F16, kind="ExternalInput").ap()
    cf_d = nc.dram_tensor("constsf", (P, CF_COLS), F32, kind="ExternalInput").ap()
    # output chunk-major: rows (c, p), cols (i, s): h8[i*128+p, c*512+s]
    out_d = nc.dram_tensor("out", (NG * P, DB * MMN), F16, kind="ExternalOutput").ap()

    with tile.TileContext(nc) as tc, ExitStack() as ctx:
        consts = ctx.enter_context(tc.tile_pool(name="consts", bufs=1))
        big = ctx.enter_context(tc.tile_pool(name="big", bufs=1))
        rp = ctx.enter_context(tc.tile_pool(name="rp", bufs=3))
        app = ctx.enter_context(tc.tile_pool(name="app", bufs=8))
        # single PSUM pool, all tags share slots: [128,1024] slot = 2 banks,
        # bufs=4 -> 8 banks
        zp = ctx.enter_context(tc.tile_pool(name="zp", bufs=4, space="PSUM"))

        # --- PE warmup on a memset dummy: starts right after the preamble
        # (no DMA dependency) and rides the HAM ramp while inputs land.
        dummy = big.tile([P, MMN], F16, name="dummy")
        nc.vector.memset(dummy[:], 0.0)
        warm = zp.tile([P, MMN], F32, name="warm", tag="z")
        for _ in range(N_WARM):
            nc.tensor.matmul(warm[:], lhsT=dummy[:, 0:P], rhs=dummy[:],
                             start=True, stop=True)

        # --- constants ---
        wb = consts.tile([P, WB_COLS], F16, name="wb")
        cf = consts.tile([P, CF_COLS], F32, name="cf")
        wint = [wb[:, i * D:(i + 1) * D] for i in range(DB)]
        wt = [wb[:, 2 * D + i * D:2 * D + (i + 1) * D] for i in range(DB)]
        identr = wb[:, WB_ID:WB_ID + P]
        bst = cf[:, 0:DB]
        at = cf[:, DB:2 * DB]

        # --- persistent buffers ---
        # x in 2 per-position-half tiles (both d-blocks each) so the first
        # half of the u projection only waits on one 0.5MB DMA
        xth = [big.tile([P, DB * WCH], F16, name=f"xth{g}") for g in range(2)]
        ut = [big.tile([P, UCOLS], F16, name=f"ut{i}") for i in range(DB)]
        hball = [big.tile([P, DB * L], F16, name=f"hb{s}") for s in range(2)]
        hb = [[hball[s][:, i * L:(i + 1) * L] for i in range(DB)]
              for s in range(2)]
        h1 = hb[1]

        # --- input DMAs. Only sync+scalar have HWDGE; each dma_start costs
        # ~800ns of descriptor-gen on its sequencer, so spread and order by
        # need: weights first (PE warmup), then cf, then x halves.
        def xdma(eng, g):
            eng.dma_start(
                xth[g][:].rearrange("p (i c) -> p i c", i=DB),
                x_d.rearrange("p (i c) -> p i c", i=DB)[
                    :, :, g * WCH:(g + 1) * WCH],
            )
        nc.sync.dma_start(wb[:], wb_d[:, :])
        nc.scalar.dma_start(cf[:], cf_d[:, :])
        xdma(nc.sync, 0)
        xdma(nc.scalar, 1)

        # u pad cols + h1 pad cols (also warms the ACT table early):
        # u[:, :7] = b*, h1[:, :7] = relu(b*)
        for j in range(DB):
            nc.scalar.activation(
                ut[j][:, 0:PAD], cf[:, 0:PAD],
                AF.Identity, bias=bst[:, j:j + 1], scale=0.0,
            )
            nc.scalar.activation(
                h1[j][:, 0:PAD], cf[:, 0:PAD],
                AF.Relu, bias=bst[:, j:j + 1], scale=0.0,
            )

        # --- u projection, wide tiles; h1 (ACT) and u (DVE) read PSUM ---
        for gw in range(2):
            for j in range(DB):
                zt = zp.tile([P, WCH], F32, name="zu", tag="z")
                for half in range(2):
                    g = 2 * gw + half
                    zh = zt[:, half * MMN:(half + 1) * MMN]
                    for i in range(DB):
                        nc.tensor.matmul(
                            zh,
                            lhsT=wint[i][:, j * P:(j + 1) * P],
                            rhs=xth[gw][:, i * WCH + half * MMN:
                                        i * WCH + half * MMN + MMN],
                            start=(i == 0),
                            stop=(i == DB - 1),
                        )
                # h1 positions [7+1024gw, min(7+1024(gw+1), 2048))
                hw = WCH if gw == 0 else WCH - PAD
                nc.scalar.activation(
                    h1[j][:, PAD + gw * WCH:PAD + gw * WCH + hw],
                    zt[:, 0:hw], AF.Relu, bias=bst[:, j:j + 1], scale=1.0,
                )
                nc.vector.tensor_scalar(
                    out=ut[j][:, PAD + gw * WCH:PAD + (gw + 1) * WCH],
                    in0=zt[:],
                    scalar1=bst[:, j:j + 1],
                    scalar2=None,
                    op0=ALU.add,
                )

        # --- steps 1..6 (wide 1024-col chunks; matmuls in 512 halves) ---
        for k in range(1, KSIZE - 1):
            hc = hb[k % 2]
            hn = hb[(k + 1) % 2]
            # a*h precomputed on DVE (tensor_scalar runs 4x on fp16 SBUF)
            # while PE does the step's matmuls; the post-relu combine is a
            # 2x tensor_tensor. scalar_tensor_tensor would run 1x.
            ahs = {}
            for c in range(NW):
                for j in range(DB):
                    ah = app.tile([P, WCH], F16, name="ah", tag="ah")
                    nc.vector.tensor_scalar(
                        out=ah[:],
                        in0=hc[j][:, c * WCH:(c + 1) * WCH],
                        scalar1=at[:, j:j + 1],
                        scalar2=None,
                        op0=ALU.mult,
                    )
                    ahs[c, j] = ah
            for c in range(NW):
                cs = c * WCH
                for j in range(DB):
                    zt = zp.tile([P, WCH], F32, name="zt", tag="z")
                    for h in range(2):
                        hs = cs + h * MMN
                        zh = zt[:, h * MMN:(h + 1) * MMN]
                        for i in range(DB):
                            nc.tensor.matmul(
                                zh,
                                lhsT=wt[i][:, j * P:(j + 1) * P],
                                rhs=hc[i][:, hs:hs + MMN],
                                start=(i == 0),
                                stop=False,
                            )
                        nc.tensor.matmul(
                            zh,
                            lhsT=identr,
                            rhs=ut[j][:, k + hs:k + hs + MMN],
                            start=False,
                            stop=True,
                        )
                    r = rp.tile([P, WCH], F16, name="r", tag="r")
                    nc.scalar.activation(r[:], zt[:], AF.Relu)
                    nc.vector.tensor_tensor(
                        out=hn[j][:, cs:cs + WCH],
                        in0=ahs[c, j][:],
                        in1=r[:],
                        op=ALU.add,
                    )

        # --- step 7 in 512-col chunks, output DMA per chunk ---
        k = KSIZE - 1
        hc = hb[k % 2]
        hn = hb[(k + 1) % 2]
        h8all = hball[(k + 1) % 2]
        ahs7 = {}
        for g in range(NG):
            for j in range(DB):
                ah = app.tile([P, MMN], F16, name="ah7", tag="ah")
                nc.vector.tensor_scalar(
                    out=ah[:],
                    in0=hc[j][:, g * MMN:(g + 1) * MMN],
                    scalar1=at[:, j:j + 1],
                    scalar2=None,
                    op0=ALU.mult,
                )
                ahs7[g, j] = ah
        for g in range(NG):
            cs = g * MMN
            for j in range(DB):
                zt = zp.tile([P, MMN], F32, name="z7", tag="z")
                for i in range(DB):
                    nc.tensor.matmul(
                        zt[:],
                        lhsT=wt[i][:, j * P:(j + 1) * P],
                        rhs=hc[i][:, cs:cs + MMN],
                        start=(i == 0),
                        stop=False,
                    )
                nc.tensor.matmul(
                    zt[:],
                    lhsT=identr,
                    rhs=ut[j][:, k + cs:k + cs + MMN],
                    start=False,
                    stop=True,
                )
                r = rp.tile([P, MMN], F16, name="r7", tag="r")
                nc.scalar.activation(r[:], zt[:], AF.Relu)
                nc.vector.tensor_tensor(
                    out=hn[j][:, cs:cs + MMN],
                    in0=ahs7[g, j][:],
                    in1=r[:],
                    op=ALU.add,
                )
            # chunk-major store: rows [g*128, (g+1)*128) of out_d, 2KB descs.
            # sync is idle in steady state; keep scalar free for relus.
            eng = nc.sync if g != 1 else nc.scalar
            eng.dma_start(
                out_d.rearrange("(c p) f -> c p f", p=P)[g]
                     .rearrange("p (i s) -> p i s", i=DB),
                h8all[:].rearrange("p (i c) -> p i c", i=DB)[
                    :, :, cs:cs + MMN],
            )

    nc.compile()
    return nc


def get_program():
    if "nc" not in _cache:
        _cache["nc"] = _build_program()
    return _cache["nc"]


def make_in_maps(x, weight, input_weight, bias, tau):
    x = np.asarray(x, dtype=np.float32)
    weight = np.asarray(weight, dtype=np.float32)
    input_weight = np.asarray(input_weight, dtype=np.float32)
    bias = np.asarray(bias, dtype=np.float32).reshape(1, D)
    tau = np.asarray(tau, dtype=np.float32).reshape(1, D)

    inv_tau = 1.0 / tau                       # (1, D)
    a = 1.0 - inv_tau
    wstar = (weight * inv_tau).astype(np.float32)          # scale columns
    winstar = (input_weight * inv_tau).astype(np.float32)
    bstar = (bias * inv_tau).astype(np.float32)
    # per-partition layout (P, DB): col j holds elems [j*P, (j+1)*P)
    bstar_t = bstar.reshape(DB, P).T
    a_t = a.reshape(DB, P).T
    ident = np.eye(P, dtype=np.float32)

    wb = np.concatenate(
        [winstar[0:P, :], winstar[P:D, :], wstar[0:P, :], wstar[P:D, :],
         ident], axis=1).astype(np.float16)
    cf = np.zeros((P, CF_COLS), np.float32)
    cf[:, 0:DB] = bstar_t
    cf[:, DB:2 * DB] = a_t

    shared = {
        "wblob": np.ascontiguousarray(wb),
        "constsf": np.ascontiguousarray(cf),
    }
    ins = []
    for b in range(NCORES):
        # xt[p, i*L + l] = x[b][l, i*128+p]
        xt = np.ascontiguousarray(
            x[b].T.reshape(DB, P, L).transpose(1, 0, 2).reshape(P, DB * L)
            .astype(np.float16))
        ins.append({"xt": xt, **shared})
    return ins


def kernel(x, weight, input_weight, bias, tau, ksize, _trace=False):
    assert int(ksize) == KSIZE
    nc = get_program()
    in_maps = make_in_maps(x, weight, input_weight, bias, tau)
    res = run_bass_kernel_spmd(
        nc, in_maps, core_ids=list(range(NCORES)), trace=_trace
    )
    outs = []
    for b in range(NCORES):
        od = np.asarray(res.results[b]["out"])  # (NG*P, DB*MMN) f16
        out_b = (od.reshape(NG, P, DB, MMN).transpose(0, 3, 2, 1)
                 .reshape(L, D))
        outs.append(out_b)
    out = np.stack(outs, axis=0)
    if _trace:
        _cache["last_results"] = res
    return out.astype(np.float32)
